# revision 4
# baseline (speedup 1.0000x reference)
"""Trainium2 Bass kernel for nn_AwkwardRNNDoubleJagged (8-core tensor-parallel LSTM).

v2 strategy
-----------
One long sequential LSTM chain of S = sum(lengths) steps (particle boundaries
reset half the state; host flattens the schedule).  Per step the dominant work
is W_hh @ h with W_hh [8192, 2048]; the 4H gate dim is sharded across the 8
NeuronCores (1024 gate rows / core, bf16, SBUF-resident).

Differences vs the ncfw-AllGather baseline:
 * h exchange is SBUF->SBUF via remote_dma_broadcast (relative XOR routing,
   one single-dest broadcast per peer), signalled with per-sender semaphores.
   No HBM staging, no ncfw collective per step.
 * All per-step bias data (b + w_in*x_t) is preloaded to SBUF once; a DVE
   copy pre-fills PSUM so the matmuls accumulate on top (start=False).
 * Particle-boundary shuffles are emitted only for boundary steps; boundary
   matvecs use only the e=0 weight half against the e=1 state half (64
   matmuls instead of 128).
 * Double-buffered h_all; core r stores hidden chunk q at slot q^r so the
   SPMD program uses only static APs (slot 0 is always the core's own slice).

Hidden layout: h_all[p, 2*slot+e] = h[e*1024 + 128*(slot^core) + p].
Gate columns: [i0, f0, o0, i1, f1, o1, g0, g1] (Xe = gate X, hidden half e).
"""
import numpy as np
import ml_dtypes

NCORES = 8
H = 2048
KERNEL_STATS = {}
GATE_OF_COL = [0, 1, 3, 0, 1, 3, 2, 2]
HALF_OF_COL = [0, 0, 0, 1, 1, 1, 0, 1]


def _build_probe():
    """Tiny program that performs the same seven single-dest XOR broadcasts
    as the main kernel, with each core sending 100 + 10*core_id.  The host
    reads back each core's slot contents to learn which peer's data lands in
    which slot (the driver may remap logical->physical NCs, which changes the
    XOR routing)."""
    import concourse.bass as bass
    import concourse.bacc as bacc
    import concourse.tile as tile
    import concourse.mybir as mybir
    _patch_birsim_off()
    BF16 = mybir.dt.bfloat16

    nc = bacc.Bacc("TRN2", target_bir_lowering=False, debug=False,
                   num_devices=NCORES)
    nc.detect_race_conditions = False
    id_dram = nc.dram_tensor("ident", [128, 2], BF16, kind="ExternalInput")
    rec_dram = nc.dram_tensor("rec", [128, 16], BF16, kind="ExternalOutput")

    with tile.TileContext(nc) as tc:
        with tc.tile_pool(name="p", bufs=1) as pool:
            hx = pool.tile([128, 16], BF16, name="hx")
            nc.vector.memset(hx[:], 0.0)
            rsem = [nc.alloc_semaphore(f"prsem{j}") for j in range(1, NCORES)]
            lsem = nc.alloc_semaphore("plsem")
            bw = nc.gpsimd.bir_kernel_barrier_wait([list(range(NCORES))])
            bw.ins.sync_info.on_wait[0].wait_value = 0
            nc.sync.dma_start(hx[:, 0:2], id_dram[:])
            for j in range(1, NCORES):
                rd = [None] * NCORES
                rd[j] = (0, j)
                nc.gpsimd.remote_dma_broadcast(
                    hx[:, 2 * j:2 * j + 2], hx[:, 0:2],
                    remote_sem=rsem[j - 1], local_sem=lsem, rdests=rd)
            nc.gpsimd.trigger_dma(count=None)
            out_dma = nc.sync.dma_start(rec_dram[:], hx[:])

    bw.ins.sync_info.on_wait[0].wait_value = nc.bir_kernel_barrier_sem_inc
    inst = out_dma.ins
    si = inst.sync_info
    ow = si.on_wait
    for j in range(1, NCORES):
        ow.append(mybir.SyncWait(sync_type="semaphore", id=rsem[j - 1].num,
                                 ant_name=rsem[j - 1].name,
                                 wait_mode="sem-ge-imm", wait_value=2))
    si.on_wait = ow
    inst.sync_info = si
    nc.compile()
    return nc


_CHUNK_AT = None


def _probe_mapping():
    """Returns CHUNK_AT[core][slot] = logical id of the core whose slice
    lands in that slot."""
    global _CHUNK_AT
    if _CHUNK_AT is not None:
        return _CHUNK_AT
    import jax
    runner = _SegRunner(_build_probe())
    ident = np.zeros((NCORES * 128, 2), ml_dtypes.bfloat16)
    for m in range(NCORES):
        ident[128 * m:128 * (m + 1)] = 100 + 10 * m
    outs = runner(dict(ident=jax.device_put(ident)))
    rec = np.asarray(outs["rec"]).astype(np.float32).reshape(NCORES, 128, 8, 2)
    chunk_at = np.zeros((NCORES, NCORES), np.int64)
    for r in range(NCORES):
        for j in range(NCORES):
            v = rec[r][0, j, 0]
            if j == 0:
                chunk_at[r][j] = r
                continue
            s = int(round((v - 100) / 10))
            if not (0 <= s < NCORES) or abs(v - (100 + 10 * s)) > 0.5:
                raise RuntimeError(
                    f"probe: core {r} slot {j} has unexpected value {v}; "
                    f"rec row: {rec[r][0, :, 0]}")
            chunk_at[r][j] = s
    KERNEL_STATS["chunk_at"] = chunk_at.tolist()
    _CHUNK_AT = chunk_at
    return chunk_at


def _host_prep(event, lengths, W_ih, W_hh, b_ih, b_hh, chunk_at=None):
    event = np.asarray(event, np.float32)
    lengths = np.asarray(lengths).astype(np.int64)
    W_hh = np.asarray(W_hh, np.float32)
    w_in = np.asarray(W_ih, np.float32)[:, 0]
    bsum = np.asarray(b_ih, np.float32) + np.asarray(b_hh, np.float32)

    xs, bnd = [], []
    for p in range(event.shape[0]):
        for t in range(int(lengths[p])):
            xs.append(event[p, t])
            bnd.append(t == 0)
    xs = np.asarray(xs, np.float32)
    S = len(xs)

    cols = np.arange(8)
    gates = np.asarray(GATE_OF_COL)[cols]
    halves = np.asarray(HALF_OF_COL)[cols]
    p_idx = np.arange(128)
    m_idx = np.arange(NCORES)
    rows = (gates[None, :, None] * 2048 + halves[None, :, None] * 1024
            + 128 * m_idx[:, None, None] + p_idx[None, None, :])  # [m, col, p]
    kc = np.arange(16)
    slots, es = kc // 2, kc % 2

    Wt_cores, PS_cores = [], []
    for m in range(NCORES):
        if chunk_at is None:
            qs = slots ^ m                  # chunk stored at slot j on core m
        else:
            qs = np.asarray(chunk_at[m])[slots]
        khid = es[:, None] * 1024 + 128 * qs[:, None] + np.arange(128)[None, :]
        g = W_hh[rows[m][:, None, None, :], khid[None, :, :, None]]
        g = np.transpose(g, (2, 0, 1, 3)).reshape(128, 8 * 16 * 128)
        Wt_cores.append(np.ascontiguousarray(g.astype(ml_dtypes.bfloat16)))
        r = rows[m]
        Bt = bsum[r][None] + w_in[r][None] * xs[:, None, None]   # [S, col, p]
        ps = np.transpose(Bt, (2, 0, 1)).reshape(128, S * 8)     # [p, 8s+col]
        PS_cores.append(np.ascontiguousarray(ps.astype(np.float32)))
    return S, bnd, Wt_cores, PS_cores


def _patch_birsim_off():
    """walrus's birsim pass simulates the whole program at compile time;
    for our ~150k-instruction program that is minutes of compile for no
    benefit.  Rebuild bir_verify_and_optimise with birsim disabled."""
    import inspect
    import concourse.bass_utils as bu
    if getattr(bu, "_birsim_patched", False):
        return
    try:
        src = inspect.getsource(bu.bir_verify_and_optimise)
    except OSError:
        return  # already redefined by someone else
    src = src.replace('"--enable-birsim=true",', '"--enable-birsim=false",')
    exec(src, bu.__dict__)
    bu._birsim_patched = True


def _build_program(S, bnd):
    import concourse.bass as bass
    import concourse.bacc as bacc
    import concourse.tile as tile
    import concourse.mybir as mybir
    _patch_birsim_off()
    F32 = mybir.dt.float32
    BF16 = mybir.dt.bfloat16
    AFT = mybir.ActivationFunctionType

    nc = bacc.Bacc("TRN2", target_bir_lowering=False, debug=False,
                   num_devices=NCORES)
    wt_dram = nc.dram_tensor("wt", [128, 8 * 16 * 128], BF16, kind="ExternalInput")
    ps_dram = nc.dram_tensor("perstep", [128, S * 8], F32, kind="ExternalInput")
    out_dram = nc.dram_tensor("h32_out", [128, 2], F32, kind="ExternalOutput")

    with tile.TileContext(nc) as tc:
        with tc.tile_pool(name="wt", bufs=1) as wtp, \
             tc.tile_pool(name="psb", bufs=1) as psp, \
             tc.tile_pool(name="state", bufs=1) as stp, \
             tc.tile_pool(name="tmp", bufs=3) as tp, \
             tc.tile_pool(name="gps", bufs=2, space="PSUM") as psum_pool:

            wt = wtp.tile([128, 8 * 16 * 128], BF16)
            nc.sync.dma_start(wt[:], wt_dram[:])
            psb = psp.tile([128, S * 8], F32)
            nc.sync.dma_start(psb[:], ps_dram[:])

            hbuf = [stp.tile([128, 16], BF16, tag="hA", name="hA"),
                    stp.tile([128, 16], BF16, tag="hB", name="hB")]
            c = stp.tile([128, 2], F32)
            h32 = stp.tile([128, 2], F32)
            # hbuf[0] is the step-0 input state (must be zero).  hbuf[1] is
            # fully written (own slice by DVE, remote slices by peer step-0
            # sends) before its first read at step 1 — memsetting it would
            # race with those sends, so don't.
            nc.vector.memset(hbuf[0][:], 0.0)
            nc.vector.memset(c[:], 0.0)

            rsem = [nc.alloc_semaphore(f"rsem{j}") for j in range(1, NCORES)]
            lsem = nc.alloc_semaphore("lsem")
            # Waits on remotely-incremented semaphores would deadlock the Tile
            # scheduling sim (it simulates one core, no remote increments),
            # and standalone dep-free wait instructions get hoisted by the
            # scheduler.  Instead, attach each wait as an extra sync condition
            # on the first matmul that reads the corresponding h_all slot —
            # added post-scheduling, right before nc.compile().
            wait_attach = []  # (BassInstruction, sem, value)

            bw = nc.gpsimd.bir_kernel_barrier_wait([list(range(NCORES))])
            bw.ins.sync_info.on_wait[0].wait_value = 0
            barrier_patch = (bw, nc.bir_kernel_barrier_sem_inc)

            def wtile(col, kcc):
                return wt[:, bass.ts(col * 16 + kcc, 128)]

            for s in range(S):
                bcur = hbuf[s % 2]
                bnxt = hbuf[(s + 1) % 2]
                psum = psum_pool.tile([128, 8], F32, tag="gates")
                nc.vector.tensor_copy(psum[:], psb[:, bass.ts(s, 8)])
                if bnd[s]:
                    # c <- [c_half1, 0]
                    nc.vector.tensor_copy(c[:, 0:1], c[:, 1:2])
                    nc.vector.memset(c[:, 1:2], 0.0)
                    # gates += W[:, e0 chunks] @ h_prev[e1 chunks]
                    mm_list = [(2 * q, 2 * q + 1) for q in range(8)]
                else:
                    mm_list = [(kcc, kcc) for kcc in range(16)]
                last = len(mm_list) - 1
                for i, (wk, hk) in enumerate(mm_list):
                    slot = hk // 2
                    for col in range(8):
                        mm = nc.tensor.matmul(psum[:, col:col + 1],
                                              wtile(col, wk),
                                              bcur[:, hk:hk + 1],
                                              start=False, stop=(i == last),
                                              skip_group_check=True)
                        if col == 0 and slot > 0 and s > 0 and (bnd[s] or hk % 2 == 0):
                            wait_attach.append((mm, rsem[slot - 1], 2 * s))

                sg = tp.tile([128, 6], F32, tag="sg")
                tg = tp.tile([128, 2], F32, tag="tg")
                nc.scalar.activation(sg[:], psum[:, 0:6], AFT.Sigmoid)
                nc.scalar.activation(tg[:], psum[:, 6:8], AFT.Tanh)
                u = tp.tile([128, 2], F32, tag="u")
                v = tp.tile([128, 2], F32, tag="v")
                nc.vector.tensor_mul(u[:], sg[:, 0:6:3], tg[:])   # i * g
                nc.vector.tensor_mul(v[:], sg[:, 1:6:3], c[:])    # f * c
                nc.vector.tensor_add(c[:], u[:], v[:])
                tct = tp.tile([128, 2], F32, tag="tct")
                nc.scalar.activation(tct[:], c[:], AFT.Tanh)
                if s < S - 1:
                    nc.vector.tensor_mul(bnxt[:, 0:2], sg[:, 2:6:3], tct[:])
                    for j in range(1, NCORES):
                        rd = [None] * NCORES
                        rd[j] = (0, j)
                        nc.gpsimd.remote_dma_broadcast(
                            bnxt[:, 2 * j:2 * j + 2], bnxt[:, 0:2],
                            remote_sem=rsem[j - 1], local_sem=lsem, rdests=rd)
                    nc.gpsimd.trigger_dma(count=None)
                else:
                    nc.vector.tensor_mul(h32[:], sg[:, 2:6:3], tct[:])
                    nc.sync.dma_start(out_dram[:], h32[:])

    bw, bval = barrier_patch
    bw.ins.sync_info.on_wait[0].wait_value = bval
    for binst, sem, val in wait_attach:
        inst = binst.ins
        w = mybir.SyncWait(sync_type="semaphore", id=sem.num, ant_name=sem.name,
                           wait_mode="sem-ge-imm", wait_value=val)
        si = inst.sync_info
        if si is None:
            inst.sync_info = mybir.SyncInfo(on_wait=[w], on_update=[])
        else:
            ow = si.on_wait
            ow.append(w)
            si.on_wait = ow
            inst.sync_info = si
    nc.compile()
    return nc


class _SegRunner:
    """Jit a compiled bass program for multi-core execution."""

    def __init__(self, nc):
        import jax
        from jax.experimental.shard_map import shard_map
        from jax.sharding import Mesh, PartitionSpec
        import concourse.mybir as mybir
        from concourse import bass2jax
        bass2jax.install_neuronx_cc_hook()
        self.jax = jax
        partition_name = nc.partition_id_tensor.name if nc.partition_id_tensor else None
        in_names, out_names, out_avals, zero_shapes = [], [], [], []
        for alloc in nc.m.functions[0].allocations:
            if not isinstance(alloc, mybir.MemoryLocationSet):
                continue
            name = alloc.memorylocations[0].name
            if alloc.kind == "ExternalInput":
                if name != partition_name:
                    in_names.append(name)
            elif alloc.kind == "ExternalOutput":
                out_names.append(name)
                shape = tuple(alloc.tensor_shape)
                dtype = mybir.dt.np(alloc.dtype)
                out_avals.append(jax.core.ShapedArray(shape, dtype))
                zero_shapes.append((shape, dtype))
        self.in_names, self.out_names = in_names, out_names
        self.zero_shapes = zero_shapes
        n_params, n_outs = len(in_names), len(out_names)

        def _body(*args):
            operands = list(args)
            if partition_name is not None:
                operands.append(bass2jax.partition_id_tensor())
            names = list(in_names) + list(out_names) + (
                [partition_name] if partition_name else [])
            outs = bass2jax._bass_exec_p.bind(
                *operands,
                out_avals=tuple(out_avals),
                in_names=tuple(names),
                out_names=tuple(out_names),
                lowering_input_output_aliases=(),
                sim_require_finite=True,
                sim_require_nnan=True,
                nc=nc,
            )
            return tuple(outs)

        devices = jax.devices()[:NCORES]
        mesh = Mesh(np.asarray(devices), ("core",))
        in_specs = (PartitionSpec("core"),) * (n_params + n_outs)
        out_specs = (PartitionSpec("core"),) * n_outs
        donate = (tuple(range(n_params, n_params + n_outs))
                  if jax.default_backend() != "cpu" else ())
        self.fn = jax.jit(
            shard_map(_body, mesh=mesh, in_specs=in_specs,
                      out_specs=out_specs, check_rep=False),
            donate_argnums=donate,
            keep_unused=True,
        )

    def __call__(self, named_inputs):
        args = [named_inputs[nm] for nm in self.in_names]
        zeros = [np.zeros((NCORES * sh[0], *sh[1:]), dt)
                 for sh, dt in self.zero_shapes]
        outs = self.fn(*args, *zeros)
        return dict(zip(self.out_names, outs))


def kernel(**inputs) -> np.ndarray:
    import time as _time
    import jax
    chunk_at = _probe_mapping()
    S, bnd, Wt_cores, PS_cores = _host_prep(**inputs, chunk_at=chunk_at)

    runner = _SegRunner(_build_program(S, bnd))
    wt_dev = jax.device_put(np.concatenate(Wt_cores, axis=0))
    ps_dev = jax.device_put(np.concatenate(PS_cores, axis=0))

    def run_once():
        t0 = _time.perf_counter()
        outs = runner(dict(wt=wt_dev, perstep=ps_dev))
        res = np.asarray(outs["h32_out"])
        return res, _time.perf_counter() - t0

    _, warm_dt = run_once()            # compile + warm
    times = []
    for _ in range(3):                 # timed passes; min damps host noise
        h32_flat, timed_dt = run_once()
        times.append(timed_dt)
    KERNEL_STATS["exec_time_ns"] = int(min(times) * 1e9)
    KERNEL_STATS["all_times_ms"] = [round(t * 1e3, 2) for t in times]
    KERNEL_STATS["warm_wall_s"] = warm_dt
    h32 = h32_flat.reshape(NCORES, 128, 2)

    h = np.zeros(H, np.float32)
    for q in range(NCORES):
        h[128 * q:128 * (q + 1)] = h32[q][:, 0]
        h[1024 + 128 * q:1024 + 128 * (q + 1)] = h32[q][:, 1]
    return h.reshape(1, 1, H)
